# revision 2
# baseline (speedup 1.0000x reference)
"""Distributed BW-Cholesky whitening block for Trainium2 (8 NeuronCores).

Strategy (data-parallel over batch, per the sharding hint):
  - Shard X [64,256,56,56] by batch: 8 batches per core.
  - NEFF A (stats): each core computes per-channel sums and per-128-channel-half
    second-moment matrices (x x^T accumulated over its shard) using PE
    transpose + matmul accumulation in PSUM.
  - Host: reduce the tiny per-core stats (the "all-reduce" of [32,8,1]/[32,8,8]),
    compute mean/cov, running-stat updates, cov fix, 32x 8x8 Cholesky and
    triangular inverse (microseconds of scalar work), and build a 128x128
    block-diagonal whitening matrix per channel half plus a fused bias
    (beta - W @ mean).
  - NEFF B (apply): each core computes Y = W_blockdiag @ x + bias as a single
    K=128 matmul per tile (memory-bound roofline), fused bias on ScalarE.
"""

import sys

if "/opt/trn_rl_repo" not in sys.path:
    sys.path.insert(0, "/opt/trn_rl_repo")

import numpy as np

from concourse import bacc, bass, mybir, tile
from concourse.bass_utils import run_bass_kernel_spmd

F32 = mybir.dt.float32

B, C, H, W = 64, 256, 56, 56
HW = H * W                      # 3136
G, CG = 32, 8                   # 32 groups of 8 channels
N_CORES = 8
B_LOC = B // N_CORES            # 8 batches per core
M_TOT = B * HW                  # 200704 samples per channel
EPS = 1e-05
MOMENTUM = 0.1
FIX_FACTOR = 0.9

PAIRS = B_LOC // 2              # process batches in pairs: 2*3136 = 6272 = 49*128
FPAIR = 2 * HW                  # 6272
NCHUNK = FPAIR // 128           # 49 transpose/matmul chunks per pair-tile
APPLY_N = 448                   # matmul free-dim chunk in apply pass (3136 = 7*448)


def _build_stats():
    nc = bacc.Bacc("TRN2", target_bir_lowering=False, debug=False, num_devices=N_CORES)
    x = nc.declare_dram_parameter("x", [B_LOC, C, HW], F32, isOutput=False)
    eye = nc.declare_dram_parameter("eye", [128, 128], F32, isOutput=False)
    sumx = nc.declare_dram_parameter("sumx", [2, 128, 1], F32, isOutput=True)
    sumxx = nc.declare_dram_parameter("sumxx", [2, 128, 128], F32, isOutput=True)

    with tile.TileContext(nc) as tc:
        with (
            tc.tile_pool(name="xin", bufs=2) as xin_pool,
            tc.tile_pool(name="xt", bufs=3) as xt_pool,
            tc.tile_pool(name="acc", bufs=1) as acc_pool,
            tc.tile_pool(name="tp", bufs=3, space=bass.MemorySpace.PSUM) as tp_pool,
            tc.tile_pool(name="mp", bufs=2, space=bass.MemorySpace.PSUM) as mp_pool,
        ):
            eye_sb = acc_pool.tile([128, 128], F32, tag="eye")
            nc.sync.dma_start(eye_sb[:], eye[:])

            sxx_sb = []
            red_all = []
            for ci in range(2):
                t = acc_pool.tile([128, 128], F32, tag=f"sxx{ci}", name=f"sxx{ci}")
                nc.gpsimd.memset(t[:], 0.0)
                sxx_sb.append(t)
                red_all.append(acc_pool.tile([128, PAIRS], F32, tag=f"red{ci}", name=f"red{ci}"))

            for ci in range(2):
                cs = ci * 128
                for p in range(PAIRS):
                    xt = xin_pool.tile([128, FPAIR], F32, tag="xin")
                    for j in range(2):
                        b = 2 * p + j
                        nc.sync.dma_start(
                            xt[:, j * HW:(j + 1) * HW], x[b, cs:cs + 128, :]
                        )
                    # per-channel partial sum for this pair-tile
                    nc.vector.reduce_sum(
                        red_all[ci][:, p:p + 1], xt[:], axis=mybir.AxisListType.X
                    )
                    # second moment: transpose each 128x128 chunk, accumulate
                    # xT.T @ xT (= x x^T over those 128 samples) into PSUM.
                    acc = mp_pool.tile([128, 128], F32, tag="mmacc")
                    for k in range(NCHUNK):
                        tp = tp_pool.tile([128, 128], F32, tag="tp")
                        nc.tensor.transpose(
                            tp[:], xt[:, k * 128:(k + 1) * 128], eye_sb[:]
                        )
                        xts = xt_pool.tile([128, 128], F32, tag="xts")
                        nc.vector.tensor_copy(xts[:], tp[:])
                        nc.tensor.matmul(
                            acc[:],
                            xts[:],
                            xts[:],
                            start=(k == 0),
                            stop=(k == NCHUNK - 1),
                        )
                    nc.vector.tensor_add(sxx_sb[ci][:], sxx_sb[ci][:], acc[:])

            for ci in range(2):
                fin = acc_pool.tile([128, 1], F32, tag=f"fin{ci}", name=f"fin{ci}")
                nc.vector.reduce_sum(fin[:], red_all[ci][:], axis=mybir.AxisListType.X)
                nc.sync.dma_start(sumx[ci], fin[:])
                nc.sync.dma_start(sumxx[ci], sxx_sb[ci][:])

    nc.compile()
    return nc


def _build_apply():
    nc = bacc.Bacc("TRN2", target_bir_lowering=False, debug=False, num_devices=N_CORES)
    x = nc.declare_dram_parameter("x", [B_LOC, C, HW], F32, isOutput=False)
    w = nc.declare_dram_parameter("w", [2, 128, 128], F32, isOutput=False)
    bias = nc.declare_dram_parameter("bias", [2, 128, 1], F32, isOutput=False)
    y = nc.declare_dram_parameter("y", [B_LOC, C, HW], F32, isOutput=True)

    with tile.TileContext(nc) as tc:
        with (
            tc.tile_pool(name="xin", bufs=3) as xin_pool,
            tc.tile_pool(name="yout", bufs=3) as yout_pool,
            tc.tile_pool(name="cst", bufs=1) as cst_pool,
            tc.tile_pool(name="ps", bufs=4, space=bass.MemorySpace.PSUM) as ps_pool,
        ):
            w_sb = []
            b_sb = []
            for ci in range(2):
                wt = cst_pool.tile([128, 128], F32, tag=f"w{ci}", name=f"w{ci}")
                nc.sync.dma_start(wt[:], w[ci])
                w_sb.append(wt)
                bt = cst_pool.tile([128, 1], F32, tag=f"b{ci}", name=f"b{ci}")
                nc.sync.dma_start(bt[:], bias[ci])
                b_sb.append(bt)

            for b in range(B_LOC):
                for ci in range(2):
                    cs = ci * 128
                    xt = xin_pool.tile([128, HW], F32, tag="xin")
                    nc.sync.dma_start(xt[:], x[b, cs:cs + 128, :])
                    yt = yout_pool.tile([128, HW], F32, tag="yout")
                    for k in range(HW // APPLY_N):
                        sl = slice(k * APPLY_N, (k + 1) * APPLY_N)
                        pt = ps_pool.tile([128, APPLY_N], F32, tag="ps")
                        nc.tensor.matmul(pt[:], w_sb[ci][:], xt[:, sl])
                        nc.scalar.activation(
                            yt[:, sl],
                            pt[:],
                            mybir.ActivationFunctionType.Identity,
                            bias=b_sb[ci][:],
                        )
                    nc.sync.dma_start(y[b, cs:cs + 128, :], yt[:])

    nc.compile()
    return nc


_CACHE = {}


def _get(name):
    if name not in _CACHE:
        _CACHE[name] = _build_stats() if name == "stats" else _build_apply()
    return _CACHE[name]


def kernel(X, running_mean, running_cov, beta):
    X = np.ascontiguousarray(np.asarray(X, dtype=np.float32))
    running_mean = np.asarray(running_mean, dtype=np.float32)
    running_cov = np.asarray(running_cov, dtype=np.float32)
    beta = np.asarray(beta, dtype=np.float32)

    x3 = X.reshape(B, C, HW)
    shards = [x3[i * B_LOC:(i + 1) * B_LOC] for i in range(N_CORES)]
    eye = np.eye(128, dtype=np.float32)
    core_ids = list(range(N_CORES))

    # ---- Pass 1: per-core partial sums ----
    nc_stats = _get("stats")
    in_maps = [{"x": s, "eye": eye} for s in shards]
    res = run_bass_kernel_spmd(nc_stats, in_maps, core_ids).results

    sum_x = np.zeros((2, 128), dtype=np.float64)
    sum_xx = np.zeros((2, 128, 128), dtype=np.float64)
    for r in res:
        sum_x += r["sumx"][:, :, 0].astype(np.float64)
        sum_xx += r["sumxx"].astype(np.float64)

    # ---- Host: tiny per-group algebra (mean/cov/chol on 32 8x8 matrices) ----
    mean_c = sum_x.reshape(C) / M_TOT                       # [256]
    mean = mean_c.reshape(G, CG, 1)                         # [32,8,1]

    cov = np.zeros((G, CG, CG), dtype=np.float64)
    for g in range(G):
        half, base = divmod(g, 16)
        blk = sum_xx[half, base * CG:(base + 1) * CG, base * CG:(base + 1) * CG]
        cov[g] = blk / M_TOT - np.outer(mean[g, :, 0], mean[g, :, 0])
    cov += EPS * np.eye(CG)[None]

    rm_new = (1.0 - MOMENTUM) * running_mean.astype(np.float64) + MOMENTUM * mean
    rc_new = (1.0 - MOMENTUM) * running_cov.astype(np.float64) + MOMENTUM * cov
    off = FIX_FACTOR * (1.0 - np.eye(CG)) + np.eye(CG)
    rc_fixed = off[None] * rc_new

    L = np.linalg.cholesky(rc_fixed)                        # [32,8,8]
    Winv = np.linalg.solve(L, np.broadcast_to(np.eye(CG), (G, CG, CG)))

    # Block-diagonal whitening matrix per 128-channel half, pre-transposed for
    # the PE (out = lhsT.T @ rhs), plus fused bias beta - W @ mean.
    wT = np.zeros((2, 128, 128), dtype=np.float32)
    bias_h = np.zeros((2, 128, 1), dtype=np.float32)
    wmean = np.einsum("gij,gj->gi", Winv, mean[:, :, 0])    # [32,8]
    for g in range(G):
        half, base = divmod(g, 16)
        sl = slice(base * CG, (base + 1) * CG)
        wT[half, sl, sl] = Winv[g].T.astype(np.float32)
        bias_h[half, sl, 0] = (
            beta.reshape(G, CG)[g].astype(np.float64) - wmean[g]
        ).astype(np.float32)

    # ---- Pass 2: apply whitening + bias ----
    nc_apply = _get("apply")
    in_maps = [{"x": s, "w": wT, "bias": bias_h} for s in shards]
    res = run_bass_kernel_spmd(nc_apply, in_maps, core_ids).results

    Y = np.empty((B, C, HW), dtype=np.float32)
    for i, r in enumerate(res):
        Y[i * B_LOC:(i + 1) * B_LOC] = r["y"]
    Y = Y.reshape(B, C, H, W)

    return (
        Y,
        rm_new.astype(np.float32),
        rc_fixed.astype(np.float32),
    )
